# revision 29
# baseline (speedup 1.0000x reference)
"""2-layer GCN (PyG GCNConv x2 + ReLU) on 8 Trainium2 NeuronCores.

out = Ahat @ relu(Ahat @ X @ W1 + b1) @ W2 + b2,  Ahat = D^-1/2 (A+I) D^-1/2

Strategy (destination-sharded, graph-parallel):
  - Host: shard destination nodes across 8 cores (2500 each); per core, sort
    incoming edges by destination, pack into 128-edge chunks per
    128-destination tile.  Segment-sum aggregation becomes PSUM-accumulated
    matmuls against one-hot fp16 selection matrices S (race-free, exact fp32
    accumulation).  The symmetric normalization is folded into the node
    features (rows pre-scaled by D^-1/2 on the host) and fused
    destination-side scales on the Activation engine.
  - S matrices are generated ON-CHIP, just-in-time per gather batch (DVE
    is_equal of an iota row against per-slot destination columns) instead of
    DMA'd: saves ~11MB of HBM/DMA traffic per core that contended with the
    edge gathers (the dominant cost: SWDGE queues process ~130 descs/us/queue
    and each edge is one descriptor).
  - Associativity: (A+I)(Xs) @ W1 aggregates the *input* features first, so
    layer-1 gathers run against the replicated input table from t=0; the
    W1/W2 matmuls run post-aggregation on each core's 2500-node shard only.
  - Both layers gather with the SAME index table (xsfull and the AllGather'd
    y2full are both in node order).
  - Per-tile epilogue scales/relu run on the Scalar (Activation) engine with
    per-partition scale operands, keeping the DVE free for S generation.
  - Device, per core: L1 aggregation (dma_gather rows of Xs + S matmuls,
    descriptor generation round-robined over SWDGE queues 0-3) -> per-tile
    epilogue (scale, PE transpose, @W1, relu-scale, transpose, @W2) ->
    AllGather y2 (compact fp16) -> L2 aggregation (same S tiles and indices)
    -> final D^-1/2 scale (+bias) -> output shard, fp32.
"""

import sys

sys.path.insert(0, "/opt/trn_rl_repo")

import numpy as np

import concourse.bacc as bacc
import concourse.tile as tile
import concourse.mybir as mybir
from concourse import bass_utils

N_CORES = 8
N_NODES = 20000
IN_CH = 256
HID_CH = 256
OUT_CH = 128
SHARD = N_NODES // N_CORES  # 2500
P = 128
N_TILES = (SHARD + P - 1) // P  # 20
GATHER_QUEUES = (0, 1, 2, 3)  # round-robin descgen across all 4 Q7 pairs
PAD_DLOC = 300.0  # out-of-range dst marker -> all-zero S row

F16 = mybir.dt.float16
F32 = mybir.dt.float32
I16 = mybir.dt.int16
AF = mybir.ActivationFunctionType


def _host_prep(doc_embeds, edge_index, W1, b1, W2, b2):
    X = np.asarray(doc_embeds, np.float32)
    ei = np.asarray(edge_index)
    src_g = ei[0].astype(np.int64)
    dst_g = ei[1].astype(np.int64)

    deg = np.bincount(dst_g, minlength=N_NODES).astype(np.float32) + 1.0
    dis = 1.0 / np.sqrt(deg)  # [N]

    xsfull = np.ascontiguousarray((X * dis[:, None]).astype(np.float16))  # [N, 256]
    W1h = np.ascontiguousarray(np.asarray(W1, np.float16))  # [256, 256]
    W2h = np.ascontiguousarray(np.asarray(W2, np.float16))  # [256, 128]

    core_of = dst_g // SHARD
    per_core = []
    counts = np.zeros((N_CORES, N_TILES), np.int64)
    loop_d = np.arange(SHARD, dtype=np.int64)
    for m in range(N_CORES):
        sel = np.nonzero(core_of == m)[0]
        # self-loops folded in as ordinary edges (same gather tables serve
        # both layers; avoids a separate per-tile DMA + identity matmul)
        s = np.concatenate([src_g[sel], m * SHARD + loop_d])
        d = np.concatenate([dst_g[sel] - m * SHARD, loop_d])
        order = np.lexsort((s, d))
        s = s[order]
        d = d[order]
        per_core.append((s, d))
        counts[m] = np.bincount(d // P, minlength=N_TILES)

    # uniform per-tile chunk counts across cores (SPMD: same program everywhere)
    C_t = np.maximum((counts.max(axis=0) + P - 1) // P, 1).astype(np.int64)
    # process tiles with many chunks first so the tail tile is cheap
    tile_order = np.argsort(-C_t, kind="stable").astype(np.int64)
    pos_of_tile = np.empty(N_TILES, np.int64)
    pos_of_tile[tile_order] = np.arange(N_TILES)
    C_sched = C_t[tile_order]
    sched_offsets = np.concatenate([[0], np.cumsum(C_sched)])
    offsets = sched_offsets[pos_of_tile]  # chunk offset per physical tile
    sumC = int(C_t.sum())
    L = sumC * P

    srcs = np.zeros((N_CORES, L), np.int64)
    dloc_all = np.full((N_CORES, P, sumC), PAD_DLOC, np.float16)
    for m in range(N_CORES):
        s, d = per_core[m]
        tile_of = d // P
        first = np.searchsorted(d, np.arange(N_TILES) * P, side="left")
        rank = np.arange(len(d)) - first[tile_of]
        pos = offsets[tile_of] * P + rank
        srcs[m, pos] = s
        dloc_all[m, pos % P, pos // P] = (d - tile_of * P).astype(np.float16)

    # wrapped int16 index layout: index i -> [16*grp + i%16, i//16], 8 replicas
    idxw = np.empty((N_CORES, P, L // 16), np.int16)
    for m in range(N_CORES):
        base = srcs[m].astype(np.int16).reshape(L // 16, 16).T  # [16, L//16]
        idxw[m] = np.tile(base, (8, 1))

    # per-partition scale tiles [128, N_TILES]: node m*SHARD + t*128 + p
    dist = np.zeros((N_CORES, P, N_TILES), np.float32)
    pad = N_TILES * P - SHARD
    for m in range(N_CORES):
        dsh = np.pad(dis[m * SHARD : (m + 1) * SHARD], (0, pad))
        dist[m] = dsh.reshape(N_TILES, P).T

    b1f = np.asarray(b1, np.float32)
    b2f = np.asarray(b2, np.float32)
    has_b1 = bool(np.any(b1f))
    has_b2 = bool(np.any(b2f))
    b1bc = np.broadcast_to(b1f, (P, HID_CH)).copy()
    b2bc = np.broadcast_to(b2f, (P, OUT_CH)).copy()

    identity = np.eye(P, dtype=np.float16)
    iotab = np.broadcast_to(np.arange(P, dtype=np.float16), (P, P)).copy()
    in_maps = []
    for m in range(N_CORES):
        im = {
            "xsfull": xsfull,
            "ident": identity,
            "iotab": iotab,
            "w1": W1h,
            "w2": W2h,
            "idxall": np.ascontiguousarray(idxw[m]),
            "dloc": np.ascontiguousarray(dloc_all[m]),
            "dist": np.ascontiguousarray(dist[m]),
        }
        if has_b1:
            im["b1bc"] = b1bc
        if has_b2:
            im["b2bc"] = b2bc
        in_maps.append(im)
    meta = dict(C_t=C_t, offsets=offsets, sumC=sumC, L=L, has_b1=has_b1,
                has_b2=has_b2, tile_order=tile_order)
    return in_maps, meta


def _build_program(meta):
    offsets = meta["offsets"]
    sumC = meta["sumC"]
    L = meta["L"]
    has_b1 = meta["has_b1"]
    has_b2 = meta["has_b2"]

    # one gather batch per scheduled tile (pad-skip via trailing -1 indices)
    C_t = meta["C_t"]
    batches = []
    batch_of_chunk = np.zeros(sumC, np.int64)
    for p, t in enumerate(meta["tile_order"]):
        c0 = int(meta["offsets"][int(t)])
        c1 = c0 + int(C_t[int(t)])
        batches.append((c0, c1))
        batch_of_chunk[c0:c1] = p
    cmax = int(C_t.max())

    nc = bacc.Bacc(
        "TRN2",
        target_bir_lowering=False,
        debug=False,
        num_devices=N_CORES,
        num_swdge_queues=4,
        dynamic_dma_scratch_size=32768,
    )

    xs_d = nc.dram_tensor("xsfull", [N_NODES, IN_CH], F16, kind="ExternalInput").ap()
    id_d = nc.dram_tensor("ident", [P, P], F16, kind="ExternalInput").ap()
    io_d = nc.dram_tensor("iotab", [P, P], F16, kind="ExternalInput").ap()
    w1 = nc.dram_tensor("w1", [IN_CH, HID_CH], F16, kind="ExternalInput").ap()
    w2 = nc.dram_tensor("w2", [HID_CH, OUT_CH], F16, kind="ExternalInput").ap()
    idx_d = nc.dram_tensor("idxall", [P, L // 16], I16, kind="ExternalInput").ap()
    dloc_d = nc.dram_tensor("dloc", [P, sumC], F16, kind="ExternalInput").ap()
    dist_d = nc.dram_tensor("dist", [P, N_TILES], F32, kind="ExternalInput").ap()
    b1_d = b2_d = None
    if has_b1:
        b1_d = nc.dram_tensor("b1bc", [P, HID_CH], F32, kind="ExternalInput").ap()
    if has_b2:
        b2_d = nc.dram_tensor("b2bc", [P, OUT_CH], F32, kind="ExternalInput").ap()
    out_d = nc.dram_tensor("out", [SHARD, OUT_CH], F32, kind="ExternalOutput").ap()

    rg = [list(range(N_CORES))]

    with tile.TileContext(nc) as tc:
        with (
            tc.tile_pool(name="dram", bufs=1, space="DRAM") as dram,
            tc.tile_pool(name="const", bufs=1) as cpool,
            tc.tile_pool(name="sseg", bufs=1) as spool,
            tc.tile_pool(name="gat", bufs=6) as gpool,
            tc.tile_pool(name="work", bufs=3) as wpool,
            tc.tile_pool(name="psa", bufs=4, space="PSUM") as ps_agg,
            tc.tile_pool(name="pst", bufs=2, space="PSUM") as ps_tr,
            tc.tile_pool(name="pso", bufs=2, space="PSUM") as ps_o,
        ):
            # ---- indices first (gathers depend only on these) ----
            idxt = cpool.tile([P, L // 16], I16)
            nc.sync.dma_start(out=idxt[:], in_=idx_d[:])

            dloct = cpool.tile([P, sumC], F16)
            nc.scalar.dma_start(out=dloct[:], in_=dloc_d[:])
            iot = cpool.tile([P, P], F16)
            nc.scalar.dma_start(out=iot[:], in_=io_d[:])

            w1t = cpool.tile([P, 2, HID_CH], F16)
            w2t = cpool.tile([P, 2, OUT_CH], F16)
            for k in range(2):
                nc.scalar.dma_start(out=w1t[:, k, :], in_=w1[k * P : (k + 1) * P, :])
                nc.scalar.dma_start(out=w2t[:, k, :], in_=w2[k * P : (k + 1) * P, :])
            distt = cpool.tile([P, N_TILES], F32)
            nc.scalar.dma_start(out=distt[:], in_=dist_d[:])
            ident = cpool.tile([P, P], F16)
            nc.scalar.dma_start(out=ident[:], in_=id_d[:])
            b1t = b2t = None
            if has_b1:
                b1t = cpool.tile([P, HID_CH], F32)
                nc.sync.dma_start(out=b1t[:], in_=b1_d[:])
            if has_b2:
                b2t = cpool.tile([P, OUT_CH], F32)
                nc.sync.dma_start(out=b2t[:], in_=b2_d[:])

            # ---- DRAM intermediates ----
            y2own = dram.tile([SHARD, OUT_CH], F16)
            y2full = dram.tile([N_NODES, OUT_CH], F16, addr_space="Shared")

            # ---- S tiles: generated on-chip, shared by both layers ----
            # S^T[e, g*128 + j] = (dloc[e, g] == j), padding dloc=300 -> zeros
            stiles = [None] * len(batches)

            def ensure_stile(b):
                if stiles[b] is not None:
                    return
                c0, c1 = batches[b]
                nch = c1 - c0
                st = spool.tile([P, nch * P], F16, name=f"sseg{b}", tag=f"sseg{b}")
                in0 = iot[:].unsqueeze(1).broadcast_to([P, nch, P])
                in1 = dloct[:, c0:c1].unsqueeze(2).broadcast_to([P, nch, P])
                nc.vector.tensor_tensor(
                    out=st[:], in0=in0, in1=in1, op=mybir.AluOpType.is_equal
                )
                stiles[b] = st

            # ---- aggregation helper (both layers) ----
            swdge_ctr = [0]  # SWDGE DMA emission counter (lane/queue pairing)

            def aggregate(src_full, width, epilogue, phase, after_pos=None):
                """Segment-sum src_full rows by destination tile via S matmuls.

                epilogue(t, tw, psum_tile) consumes the [128, width] fp32 sums.
                after_pos: optional {schedule_position: callback} hooks.
                """
                gts = {}

                def ensure_batch(b):
                    if b in gts:
                        return
                    ensure_stile(b)
                    c0, c1 = batches[b]
                    nch = c1 - c0
                    gt = gpool.tile(
                        [P, cmax, width], F16, name=f"g{phase}_{b}", tag="gat"
                    )
                    # Tile hands SWDGE DMAs their DMASW sem lane round-robin
                    # (mod 8) in program order; keep queue = emission mod 4 so
                    # every lane is only ever fed from one queue (lane FIFO
                    # order == completion order, no cross-queue sem races).
                    q = GATHER_QUEUES[swdge_ctr[0] % len(GATHER_QUEUES)]
                    swdge_ctr[0] += 1
                    nc.gpsimd.dma_gather(
                        out_ap=gt[:, :nch, :],
                        in_ap=src_full[:],
                        idxs_ap=idxt[:, c0 * 8 : c1 * 8],
                        num_idxs=nch * P,
                        num_idxs_reg=nch * P,
                        elem_size=width,
                        single_packet=False,
                        queue_num=q,
                    )
                    gts[b] = gt

                for pos, t in enumerate(meta["tile_order"]):
                    t = int(t)
                    n0 = t * P
                    tw = min(P, SHARD - n0)
                    ps = ps_agg.tile([P, HID_CH], F32, name="psagg", tag="psagg")
                    g0 = int(offsets[t])
                    g1 = g0 + int(meta["C_t"][t])
                    for g in range(g0, g1):
                        b = int(batch_of_chunk[g])
                        ensure_batch(b)
                        gl = g - batches[b][0]
                        nc.tensor.matmul(
                            ps[:, :width],
                            lhsT=stiles[b][:, gl * P : (gl + 1) * P],
                            rhs=gts[b][:, gl, :],
                            start=(g == g0),
                            stop=(g == g1 - 1),
                        )
                    epilogue(t, tw, ps)
                    if after_pos and pos in after_pos:
                        after_pos[pos]()

            # transpose [128, 2*P_cols] fp16 SBUF tile -> [128, n_k, P] fp16
            def transpose2(x_sb, n_k, name):
                xT = wpool.tile([P, n_k, P], F16, name=name, tag=name)
                for k in range(n_k):
                    pst = ps_tr.tile([P, P], F16, name="pst", tag="pst")
                    nc.tensor.transpose(
                        out=pst[:], in_=x_sb[:, k * P : (k + 1) * P], identity=ident[:]
                    )
                    nc.vector.tensor_copy(out=xT[:, k, :], in_=pst[:])
                return xT

            # ---- phase B: L1 aggregation -> out1 -> x1s -> y2 ----
            def epilogue1(t, tw, ps):
                # u = dis * aggX   [128, 256] fp16  (scalar engine, per-part scale)
                u_sb = wpool.tile([P, IN_CH], F16, name="u_sb", tag="u_sb")
                nc.scalar.activation(
                    out=u_sb[:], in_=ps[:, :IN_CH], func=AF.Copy,
                    scale=distt[:, t : t + 1],
                )
                uT = transpose2(u_sb, 2, "uT")
                pso1 = ps_o.tile([P, HID_CH], F32, name="pso1", tag="pso")
                for k in range(2):
                    nc.tensor.matmul(
                        pso1[:],
                        lhsT=uT[:, k, :],
                        rhs=w1t[:, k, :],
                        start=(k == 0),
                        stop=(k == 1),
                    )
                # x1s = dis * relu(out1 + b1) = relu(dis*out1 + dis*b1)
                x1s = wpool.tile([P, HID_CH], F16, name="x1s", tag="x1s")
                if not has_b1:
                    nc.scalar.activation(
                        out=x1s[:], in_=pso1[:], func=AF.Relu,
                        scale=distt[:, t : t + 1],
                    )
                else:
                    tmp = wpool.tile([P, HID_CH], F32, name="tmpb1", tag="tmpb1")
                    nc.vector.tensor_tensor(
                        out=tmp[:], in0=pso1[:], in1=b1t[:], op=mybir.AluOpType.add
                    )
                    nc.scalar.activation(
                        out=x1s[:], in_=tmp[:], func=AF.Relu,
                        scale=distt[:, t : t + 1],
                    )
                x1sT = transpose2(x1s, 2, "x1sT")
                psy2 = ps_o.tile([P, OUT_CH], F32, name="psy2", tag="pso")
                for k in range(2):
                    nc.tensor.matmul(
                        psy2[:],
                        lhsT=x1sT[:, k, :],
                        rhs=w2t[:, k, :],
                        start=(k == 0),
                        stop=(k == 1),
                    )
                y2sb = wpool.tile([P, OUT_CH], F16, name="y2sb", tag="y2sb")
                nc.scalar.activation(out=y2sb[:tw, :], in_=psy2[:tw, :], func=AF.Copy)
                n0 = t * P
                nc.sync.dma_start(out=y2own[n0 : n0 + tw, :], in_=y2sb[:tw, :])

            aggregate(xs_d, IN_CH, epilogue1, phase=0)
            nc.gpsimd.collective_compute(
                "AllGather",
                mybir.AluOpType.bypass,
                replica_groups=rg,
                ins=[y2own.opt()],
                outs=[y2full.opt()],
            )

            # ---- phase C: L2 aggregation -> out ----
            def epilogue2(t, tw, ps):
                outsb = wpool.tile([P, OUT_CH], F32, name="outsb", tag="outsb")
                nc.scalar.activation(
                    out=outsb[:], in_=ps[:, :OUT_CH], func=AF.Copy,
                    scale=distt[:, t : t + 1],
                )
                if has_b2:
                    nc.vector.tensor_tensor(
                        out=outsb[:], in0=outsb[:], in1=b2t[:], op=mybir.AluOpType.add
                    )
                n0 = t * P
                nc.sync.dma_start(out=out_d[n0 : n0 + tw, :], in_=outsb[:tw, :])

            aggregate(y2full, OUT_CH, epilogue2, phase=1)

    nc.compile()
    return nc


def run(inputs, trace=False, trace_kwargs=None):
    """Build, run on 8 cores, return (output, BassKernelResults)."""
    in_maps, meta = _host_prep(**inputs)
    nc = _build_program(meta)
    res = bass_utils.run_bass_kernel_spmd(
        nc,
        in_maps,
        core_ids=list(range(N_CORES)),
        trace=trace,
        **(trace_kwargs or {}),
    )
    out = np.concatenate([res.results[m]["out"] for m in range(N_CORES)], axis=0)
    return out, res


def kernel(**inputs) -> np.ndarray:
    out, _ = run(inputs)
    return out


# revision 30
# speedup vs baseline: 1.1396x; 1.1396x over previous
"""2-layer GCN (PyG GCNConv x2 + ReLU) on 8 Trainium2 NeuronCores.

out = Ahat @ relu(Ahat @ X @ W1 + b1) @ W2 + b2,  Ahat = D^-1/2 (A+I) D^-1/2

Strategy (destination-sharded, graph-parallel):
  - Host: shard destination nodes across 8 cores (2500 each); per core, sort
    incoming edges by destination, pack into 128-edge chunks per
    128-destination tile.  Segment-sum aggregation becomes PSUM-accumulated
    matmuls against one-hot fp16 selection matrices S (race-free, exact fp32
    accumulation).  The symmetric normalization is folded into the node
    features (rows pre-scaled by D^-1/2 on the host) and fused
    destination-side scales on the Activation engine.
  - S matrices are generated ON-CHIP, just-in-time per gather batch (DVE
    is_equal of an iota row against per-slot destination columns) instead of
    DMA'd: saves ~11MB of HBM/DMA traffic per core that contended with the
    edge gathers (the dominant cost: SWDGE queues process ~130 descs/us/queue
    and each edge is one descriptor).
  - Associativity: (A+I)(Xs) @ W1 aggregates the *input* features first, so
    layer-1 gathers run against the replicated input table from t=0; the
    W1/W2 matmuls run post-aggregation on each core's 2500-node shard only.
  - Both layers gather with the SAME index table (xsfull and the AllGather'd
    y2full are both in node order).
  - Per-tile epilogue scales/relu run on the Scalar (Activation) engine with
    per-partition scale operands, keeping the DVE free for S generation.
  - Device, per core: L1 aggregation (dma_gather rows of Xs + S matmuls,
    descriptor generation round-robined over SWDGE queues 0-3) -> per-tile
    epilogue (scale, PE transpose, @W1, relu-scale, transpose, @W2) ->
    AllGather y2 (compact fp16) -> L2 aggregation (same S tiles and indices)
    -> final D^-1/2 scale (+bias) -> output shard, fp32.
"""

import sys

sys.path.insert(0, "/opt/trn_rl_repo")

import numpy as np

import concourse.bacc as bacc
import concourse.tile as tile
import concourse.mybir as mybir
from concourse import bass_utils

N_CORES = 8
N_NODES = 20000
IN_CH = 256
HID_CH = 256
OUT_CH = 128
SHARD = N_NODES // N_CORES  # 2500
P = 128
N_TILES = (SHARD + P - 1) // P  # 20
GATHER_QUEUES = (0, 1, 2, 3)  # round-robin descgen across all 4 Q7 pairs
PAD_DLOC = 300.0  # out-of-range dst marker -> all-zero S row

F16 = mybir.dt.float16
F32 = mybir.dt.float32
I16 = mybir.dt.int16
AF = mybir.ActivationFunctionType


def _host_prep(doc_embeds, edge_index, W1, b1, W2, b2):
    X = np.asarray(doc_embeds, np.float32)
    ei = np.asarray(edge_index)
    src_g = ei[0].astype(np.int64)
    dst_g = ei[1].astype(np.int64)

    deg = np.bincount(dst_g, minlength=N_NODES).astype(np.float32) + 1.0
    dis = 1.0 / np.sqrt(deg)  # [N]

    xsfull = np.ascontiguousarray((X * dis[:, None]).astype(np.float16))  # [N, 256]
    W1h = np.ascontiguousarray(np.asarray(W1, np.float16))  # [256, 256]
    W2h = np.ascontiguousarray(np.asarray(W2, np.float16))  # [256, 128]

    core_of = dst_g // SHARD
    per_core = []
    counts = np.zeros((N_CORES, N_TILES), np.int64)
    loop_d = np.arange(SHARD, dtype=np.int64)
    for m in range(N_CORES):
        sel = np.nonzero(core_of == m)[0]
        # self-loops folded in as ordinary edges (same gather tables serve
        # both layers; avoids a separate per-tile DMA + identity matmul)
        s = np.concatenate([src_g[sel], m * SHARD + loop_d])
        d = np.concatenate([dst_g[sel] - m * SHARD, loop_d])
        order = np.lexsort((s, d))
        s = s[order]
        d = d[order]
        per_core.append((s, d))
        counts[m] = np.bincount(d // P, minlength=N_TILES)

    # uniform per-tile chunk counts across cores (SPMD: same program everywhere)
    C_t = np.maximum((counts.max(axis=0) + P - 1) // P, 1).astype(np.int64)
    # process tiles with many chunks first so the tail tile is cheap
    tile_order = np.argsort(-C_t, kind="stable").astype(np.int64)
    pos_of_tile = np.empty(N_TILES, np.int64)
    pos_of_tile[tile_order] = np.arange(N_TILES)
    C_sched = C_t[tile_order]
    sched_offsets = np.concatenate([[0], np.cumsum(C_sched)])
    offsets = sched_offsets[pos_of_tile]  # chunk offset per physical tile
    sumC = int(C_t.sum())
    L = sumC * P

    srcs = np.zeros((N_CORES, L), np.int64)
    dloc_all = np.full((N_CORES, P, sumC), PAD_DLOC, np.float16)
    for m in range(N_CORES):
        s, d = per_core[m]
        tile_of = d // P
        first = np.searchsorted(d, np.arange(N_TILES) * P, side="left")
        rank = np.arange(len(d)) - first[tile_of]
        pos = offsets[tile_of] * P + rank
        srcs[m, pos] = s
        dloc_all[m, pos % P, pos // P] = (d - tile_of * P).astype(np.float16)

    # wrapped int16 index layout: index i -> [16*grp + i%16, i//16], 8 replicas
    idxw = np.empty((N_CORES, P, L // 16), np.int16)
    for m in range(N_CORES):
        base = srcs[m].astype(np.int16).reshape(L // 16, 16).T  # [16, L//16]
        idxw[m] = np.tile(base, (8, 1))

    # per-partition scale tiles [128, N_TILES]: node m*SHARD + t*128 + p
    dist = np.zeros((N_CORES, P, N_TILES), np.float32)
    pad = N_TILES * P - SHARD
    for m in range(N_CORES):
        dsh = np.pad(dis[m * SHARD : (m + 1) * SHARD], (0, pad))
        dist[m] = dsh.reshape(N_TILES, P).T

    b1f = np.asarray(b1, np.float32)
    b2f = np.asarray(b2, np.float32)
    has_b1 = bool(np.any(b1f))
    has_b2 = bool(np.any(b2f))
    b1bc = np.broadcast_to(b1f, (P, HID_CH)).copy()
    b2bc = np.broadcast_to(b2f, (P, OUT_CH)).copy()

    identity = np.eye(P, dtype=np.float16)
    iotab = np.broadcast_to(np.arange(P, dtype=np.float16), (P, P)).copy()
    in_maps = []
    for m in range(N_CORES):
        im = {
            "xsfull": xsfull,
            "ident": identity,
            "iotab": iotab,
            "w1": W1h,
            "w2": W2h,
            "idxall": np.ascontiguousarray(idxw[m]),
            "dloc": np.ascontiguousarray(dloc_all[m]),
            "dist": np.ascontiguousarray(dist[m]),
        }
        if has_b1:
            im["b1bc"] = b1bc
        if has_b2:
            im["b2bc"] = b2bc
        in_maps.append(im)
    meta = dict(C_t=C_t, offsets=offsets, sumC=sumC, L=L, has_b1=has_b1,
                has_b2=has_b2, tile_order=tile_order)
    return in_maps, meta


def _build_program(meta):
    offsets = meta["offsets"]
    sumC = meta["sumC"]
    L = meta["L"]
    has_b1 = meta["has_b1"]
    has_b2 = meta["has_b2"]

    BATCH_CHUNKS = 8  # edge chunks per dma_gather call (1024 rows)
    batches = []
    c = 0
    while c < sumC:
        size = min(BATCH_CHUNKS, sumC - c)
        batches.append((c, c + size))
        c += size
    batch_of_chunk = np.zeros(sumC, np.int64)
    for bi, (c0, c1) in enumerate(batches):
        batch_of_chunk[c0:c1] = bi
    cmax = BATCH_CHUNKS

    nc = bacc.Bacc(
        "TRN2",
        target_bir_lowering=False,
        debug=False,
        num_devices=N_CORES,
        num_swdge_queues=4,
        dynamic_dma_scratch_size=32768,
    )

    xs_d = nc.dram_tensor("xsfull", [N_NODES, IN_CH], F16, kind="ExternalInput").ap()
    id_d = nc.dram_tensor("ident", [P, P], F16, kind="ExternalInput").ap()
    io_d = nc.dram_tensor("iotab", [P, P], F16, kind="ExternalInput").ap()
    w1 = nc.dram_tensor("w1", [IN_CH, HID_CH], F16, kind="ExternalInput").ap()
    w2 = nc.dram_tensor("w2", [HID_CH, OUT_CH], F16, kind="ExternalInput").ap()
    idx_d = nc.dram_tensor("idxall", [P, L // 16], I16, kind="ExternalInput").ap()
    dloc_d = nc.dram_tensor("dloc", [P, sumC], F16, kind="ExternalInput").ap()
    dist_d = nc.dram_tensor("dist", [P, N_TILES], F32, kind="ExternalInput").ap()
    b1_d = b2_d = None
    if has_b1:
        b1_d = nc.dram_tensor("b1bc", [P, HID_CH], F32, kind="ExternalInput").ap()
    if has_b2:
        b2_d = nc.dram_tensor("b2bc", [P, OUT_CH], F32, kind="ExternalInput").ap()
    out_d = nc.dram_tensor("out", [SHARD, OUT_CH], F32, kind="ExternalOutput").ap()

    rg = [list(range(N_CORES))]

    with tile.TileContext(nc) as tc:
        with (
            tc.tile_pool(name="dram", bufs=1, space="DRAM") as dram,
            tc.tile_pool(name="const", bufs=1) as cpool,
            tc.tile_pool(name="sseg", bufs=1) as spool,
            tc.tile_pool(name="gat", bufs=12) as gpool,
            tc.tile_pool(name="work", bufs=2) as wpool,
            tc.tile_pool(name="psa", bufs=4, space="PSUM") as ps_agg,
            tc.tile_pool(name="pst", bufs=2, space="PSUM") as ps_tr,
            tc.tile_pool(name="pso", bufs=2, space="PSUM") as ps_o,
        ):
            # ---- indices first (gathers depend only on these) ----
            idxt = cpool.tile([P, L // 16], I16)
            nc.sync.dma_start(out=idxt[:], in_=idx_d[:])

            dloct = cpool.tile([P, sumC], F16)
            nc.scalar.dma_start(out=dloct[:], in_=dloc_d[:])
            iot = cpool.tile([P, P], F16)
            nc.scalar.dma_start(out=iot[:], in_=io_d[:])

            w1t = cpool.tile([P, 2, HID_CH], F16)
            w2t = cpool.tile([P, 2, OUT_CH], F16)
            for k in range(2):
                nc.scalar.dma_start(out=w1t[:, k, :], in_=w1[k * P : (k + 1) * P, :])
                nc.scalar.dma_start(out=w2t[:, k, :], in_=w2[k * P : (k + 1) * P, :])
            distt = cpool.tile([P, N_TILES], F32)
            nc.scalar.dma_start(out=distt[:], in_=dist_d[:])
            ident = cpool.tile([P, P], F16)
            nc.scalar.dma_start(out=ident[:], in_=id_d[:])
            b1t = b2t = None
            if has_b1:
                b1t = cpool.tile([P, HID_CH], F32)
                nc.sync.dma_start(out=b1t[:], in_=b1_d[:])
            if has_b2:
                b2t = cpool.tile([P, OUT_CH], F32)
                nc.sync.dma_start(out=b2t[:], in_=b2_d[:])

            # ---- DRAM intermediates ----
            y2own = dram.tile([SHARD, OUT_CH], F16)
            y2full = dram.tile([N_NODES, OUT_CH], F16, addr_space="Shared")

            # ---- S tiles: generated on-chip, shared by both layers ----
            # S^T[e, g*128 + j] = (dloc[e, g] == j), padding dloc=300 -> zeros
            stiles = [None] * len(batches)

            def ensure_stile(b):
                if stiles[b] is not None:
                    return
                c0, c1 = batches[b]
                nch = c1 - c0
                st = spool.tile([P, nch * P], F16, name=f"sseg{b}", tag=f"sseg{b}")
                in0 = iot[:].unsqueeze(1).broadcast_to([P, nch, P])
                in1 = dloct[:, c0:c1].unsqueeze(2).broadcast_to([P, nch, P])
                nc.vector.tensor_tensor(
                    out=st[:], in0=in0, in1=in1, op=mybir.AluOpType.is_equal
                )
                stiles[b] = st

            # ---- aggregation helper (both layers) ----
            swdge_ctr = [0]  # SWDGE DMA emission counter (lane/queue pairing)

            def aggregate(src_full, width, epilogue, phase, after_pos=None):
                """Segment-sum src_full rows by destination tile via S matmuls.

                epilogue(t, tw, psum_tile) consumes the [128, width] fp32 sums.
                after_pos: optional {schedule_position: callback} hooks.
                """
                gts = {}

                def ensure_batch(b):
                    if b in gts:
                        return
                    ensure_stile(b)
                    c0, c1 = batches[b]
                    nch = c1 - c0
                    gt = gpool.tile(
                        [P, cmax, width], F16, name=f"g{phase}_{b}", tag="gat"
                    )
                    # Tile hands SWDGE DMAs their DMASW sem lane round-robin
                    # (mod 8) in program order; keep queue = emission mod 4 so
                    # every lane is only ever fed from one queue (lane FIFO
                    # order == completion order, no cross-queue sem races).
                    q = GATHER_QUEUES[swdge_ctr[0] % len(GATHER_QUEUES)]
                    swdge_ctr[0] += 1
                    nc.gpsimd.dma_gather(
                        out_ap=gt[:, :nch, :],
                        in_ap=src_full[:],
                        idxs_ap=idxt[:, c0 * 8 : c1 * 8],
                        num_idxs=nch * P,
                        num_idxs_reg=nch * P,
                        elem_size=width,
                        single_packet=False,
                        queue_num=q,
                    )
                    gts[b] = gt

                for pos, t in enumerate(meta["tile_order"]):
                    t = int(t)
                    n0 = t * P
                    tw = min(P, SHARD - n0)
                    ps = ps_agg.tile([P, HID_CH], F32, name="psagg", tag="psagg")
                    g0 = int(offsets[t])
                    g1 = g0 + int(meta["C_t"][t])
                    for g in range(g0, g1):
                        b = int(batch_of_chunk[g])
                        ensure_batch(b)
                        gl = g - batches[b][0]
                        nc.tensor.matmul(
                            ps[:, :width],
                            lhsT=stiles[b][:, gl * P : (gl + 1) * P],
                            rhs=gts[b][:, gl, :],
                            start=(g == g0),
                            stop=(g == g1 - 1),
                        )
                    epilogue(t, tw, ps)
                    if after_pos and pos in after_pos:
                        after_pos[pos]()

            # transpose [128, 2*P_cols] fp16 SBUF tile -> [128, n_k, P] fp16
            def transpose2(x_sb, n_k, name):
                xT = wpool.tile([P, n_k, P], F16, name=name, tag=name)
                for k in range(n_k):
                    pst = ps_tr.tile([P, P], F16, name="pst", tag="pst")
                    nc.tensor.transpose(
                        out=pst[:], in_=x_sb[:, k * P : (k + 1) * P], identity=ident[:]
                    )
                    nc.vector.tensor_copy(out=xT[:, k, :], in_=pst[:])
                return xT

            # ---- phase B: L1 aggregation -> out1 -> x1s -> y2 ----
            def epilogue1(t, tw, ps):
                # u = dis * aggX   [128, 256] fp16  (scalar engine, per-part scale)
                u_sb = wpool.tile([P, IN_CH], F16, name="u_sb", tag="u_sb")
                nc.scalar.activation(
                    out=u_sb[:], in_=ps[:, :IN_CH], func=AF.Copy,
                    scale=distt[:, t : t + 1],
                )
                uT = transpose2(u_sb, 2, "uT")
                pso1 = ps_o.tile([P, HID_CH], F32, name="pso1", tag="pso")
                for k in range(2):
                    nc.tensor.matmul(
                        pso1[:],
                        lhsT=uT[:, k, :],
                        rhs=w1t[:, k, :],
                        start=(k == 0),
                        stop=(k == 1),
                    )
                # x1s = dis * relu(out1 + b1) = relu(dis*out1 + dis*b1)
                x1s = wpool.tile([P, HID_CH], F16, name="x1s", tag="x1s")
                if not has_b1:
                    nc.scalar.activation(
                        out=x1s[:], in_=pso1[:], func=AF.Relu,
                        scale=distt[:, t : t + 1],
                    )
                else:
                    tmp = wpool.tile([P, HID_CH], F32, name="tmpb1", tag="tmpb1")
                    nc.vector.tensor_tensor(
                        out=tmp[:], in0=pso1[:], in1=b1t[:], op=mybir.AluOpType.add
                    )
                    nc.scalar.activation(
                        out=x1s[:], in_=tmp[:], func=AF.Relu,
                        scale=distt[:, t : t + 1],
                    )
                x1sT = transpose2(x1s, 2, "x1sT")
                psy2 = ps_o.tile([P, OUT_CH], F32, name="psy2", tag="pso")
                for k in range(2):
                    nc.tensor.matmul(
                        psy2[:],
                        lhsT=x1sT[:, k, :],
                        rhs=w2t[:, k, :],
                        start=(k == 0),
                        stop=(k == 1),
                    )
                y2sb = wpool.tile([P, OUT_CH], F16, name="y2sb", tag="y2sb")
                nc.scalar.activation(out=y2sb[:tw, :], in_=psy2[:tw, :], func=AF.Copy)
                n0 = t * P
                nc.sync.dma_start(out=y2own[n0 : n0 + tw, :], in_=y2sb[:tw, :])

            aggregate(xs_d, IN_CH, epilogue1, phase=0)
            nc.gpsimd.collective_compute(
                "AllGather",
                mybir.AluOpType.bypass,
                replica_groups=rg,
                ins=[y2own.opt()],
                outs=[y2full.opt()],
            )

            # ---- phase C: L2 aggregation -> out ----
            def epilogue2(t, tw, ps):
                outsb = wpool.tile([P, OUT_CH], F32, name="outsb", tag="outsb")
                nc.scalar.activation(
                    out=outsb[:], in_=ps[:, :OUT_CH], func=AF.Copy,
                    scale=distt[:, t : t + 1],
                )
                if has_b2:
                    nc.vector.tensor_tensor(
                        out=outsb[:], in0=outsb[:], in1=b2t[:], op=mybir.AluOpType.add
                    )
                n0 = t * P
                nc.sync.dma_start(out=out_d[n0 : n0 + tw, :], in_=outsb[:tw, :])

            aggregate(y2full, OUT_CH, epilogue2, phase=1)

    nc.compile()
    return nc


def run(inputs, trace=False, trace_kwargs=None):
    """Build, run on 8 cores, return (output, BassKernelResults)."""
    in_maps, meta = _host_prep(**inputs)
    nc = _build_program(meta)
    res = bass_utils.run_bass_kernel_spmd(
        nc,
        in_maps,
        core_ids=list(range(N_CORES)),
        trace=trace,
        **(trace_kwargs or {}),
    )
    out = np.concatenate([res.results[m]["out"] for m in range(N_CORES)], axis=0)
    return out, res


def kernel(**inputs) -> np.ndarray:
    out, _ = run(inputs)
    return out
